# revision 1
# baseline (speedup 1.0000x reference)
"""Deformable Conv2d (3x3, pad=1, stride=1) on Trainium2 — Bass/Tile kernel.

Sharding: data-parallel over batch across 8 NeuronCores (B=8 -> 1 image/core);
weights replicated. Per-core pipeline (all 16-bit work in fp16):
  prologue: cast x to fp16 into a zero-padded SBUF image (66-wide grid); PE
            row-transposes build a [pixel, channel] fp16 copy of x in DRAM
            with guard rows for the pair-gather windows.
  per-chunk prep (1024 pixels, pipelined ahead of the main loop):
    phase A: offset conv (18ch 3x3) as PSUM-accumulated PE matmuls with
             contiguous rhs windows over the padded-66 grid.
    phase B: bilinear coords -> corner weights + pair-gather indices on DVE
             in a pixel-major layout (partition = pixel%128); floor() via the
             1.5*2^23 magic-add trick; OOB corners get zero weight (matches
             the reference's zero-pad semantics).
  per-chunk main loop:
    per (tap, y-corner): SWDGE dma_gather of [x0,x0+1] pixel pairs (1KB
    descriptors, pixel-major); fused tensor_scalar/scalar_tensor_tensor
    combine (4 ops per 128-pixel group, per-partition weight scalars); PE
    transposes to channel-major; main conv as PSUM-accumulated fp16 matmuls
    (contraction = (channel, tap), 36 accumulation steps).
"""
import sys

sys.path.insert(0, "/opt/trn_rl_repo")

import numpy as np
import ml_dtypes

import concourse.mybir as mybir
from concourse import bacc
from concourse import bass_utils
from concourse.tile import TileContext
from concourse.tile_rust import add_dep_helper
from concourse.bass_types import AP
from concourse.masks import make_identity

B, C, O, H, W = 8, 256, 256, 64, 64
HW = H * W                  # 4096
NCORES = 8
NCHUNK = 4                  # pixel chunks in the main loop
CH = HW // NCHUNK           # 1024 pixels / chunk
JG = CH // 128              # 8 j-groups of 128 pixels / chunk
CR = H // NCHUNK            # 16 image rows / chunk
W2 = W + 2                  # padded row width (66)
NROWS = HW + 4              # xt rows: 1 guard + 4096 pixels + 3 guard
MAGIC = 12582912.0          # 1.5 * 2^23: float32 round-to-int bias
AluOp = mybir.AluOpType


def _emit(nc):
    f32, f16, i16 = mybir.dt.float32, mybir.dt.float16, mybir.dt.int16

    x_in = nc.dram_tensor("x", [C, H * W], f32, kind="ExternalInput")
    offw = nc.dram_tensor("offw", [2, 128, 9, 18], f32, kind="ExternalInput")
    offb = nc.dram_tensor("offb", [18, 1], f32, kind="ExternalInput")
    convw = nc.dram_tensor("convw", [128, 18, 256], f32, kind="ExternalInput")
    kgrid_d = nc.dram_tensor("kgrid", [128, 32, 18], f32, kind="ExternalInput")
    y_out = nc.dram_tensor("y", [O, H * W], f32, kind="ExternalOutput")
    xt_g = nc.dram_tensor("xt_g", [NROWS, 256], f16, kind="Internal")

    with TileContext(nc) as tc:
        with tc.tile_pool(name="consts", bufs=1) as consts, \
             tc.tile_pool(name="sb", bufs=2) as sb, \
             tc.tile_pool(name="pb", bufs=1) as pb, \
             tc.tile_pool(name="gather", bufs=3) as gp, \
             tc.tile_pool(name="outp", bufs=2) as op_pool, \
             tc.tile_pool(name="ps_a", bufs=2, space="PSUM") as ps_a, \
             tc.tile_pool(name="ps_tp", bufs=2, space="PSUM") as ps_tp, \
             tc.tile_pool(name="ps_acc", bufs=1, space="PSUM") as ps_acc:
            # PSUM banks: accs 4 + stp 2 + pa/offt (shared tag) 2 = 8

            # ---- constants / weights to SBUF ----
            ident = consts.tile([128, 128], f16)
            make_identity(nc, ident)
            ident_f32 = consts.tile([128, 128], f32)
            make_identity(nc, ident_f32)
            offw_sb = consts.tile([128, 2, 9, 18], f16)
            nc.gpsimd.dma_start(out=offw_sb[:],
                                in_=offw.ap().rearrange("cc c k j -> c cc k j"))
            convw_sb = consts.tile([128, 18, 256], f16)
            nc.gpsimd.dma_start(out=convw_sb[:], in_=convw.ap())
            offb_sb = consts.tile([18, 1], f32)
            nc.sync.dma_start(out=offb_sb[:], in_=offb.ap())
            kgrid = consts.tile([128, 32, 18], f32)
            nc.sync.dma_start(out=kgrid[:], in_=kgrid_d.ap())

            # ---- padded fp16 image on the 66-wide grid ----
            # x_pad[c, cc, 1+h, 1+w]; one extra zero row at the bottom so the
            # contiguous phase-A windows of the last tile stay in bounds.
            x_pad = pb.tile([128, 2, H + 3, W2], f16)
            # zero only the pad borders (keeps the x load off a full-tile
            # memset's critical path)
            nc.gpsimd.memset(x_pad[:, :, 0, :], 0.0)
            nc.gpsimd.memset(x_pad[:, :, H + 1:H + 3, :], 0.0)
            nc.gpsimd.memset(x_pad[:, :, 1:H + 1, 0], 0.0)
            nc.gpsimd.memset(x_pad[:, :, 1:H + 1, W + 1], 0.0)
            for cc in range(2):
                nc.gpsimd.dma_start(
                    out=x_pad[:, cc, 1:H + 1, 1:W + 1],
                    in_=x_in.ap()[cc * 128:(cc + 1) * 128, :].rearrange(
                        "c (h w) -> c h w", h=H))
            x_flat = x_pad.rearrange("c cc h w -> c cc (h w)")

            # ================= per-chunk prep =================
            def phase_a(c):
                """Offset conv for image rows [16c, 16c+16) -> off66_c."""
                off66 = pb.tile([18, CR * W2], f32, name=f"off66_{c}",
                                tag=f"off66_{c}")
                for t, (r0, rows) in enumerate(((0, 7), (7, 7), (14, 2))):
                    n = rows * W2
                    pa = ps_a.tile([18, 462], f32, tag="pa", name="pa")
                    for k in range(9):
                        ky, kx = k // 3, k % 3
                        base = (c * CR + r0 + ky) * W2 + kx
                        for cc in range(2):
                            nc.tensor.matmul(
                                pa[:, 0:n],
                                offw_sb[:, cc, k, :],
                                x_flat[:, cc, base:base + n],
                                start=(k == 0 and cc == 0),
                                stop=(k == 8 and cc == 1))
                    nc.vector.tensor_scalar(
                        out=off66[:, r0 * W2:r0 * W2 + n], in0=pa[:, 0:n],
                        scalar1=offb_sb[:, 0:1], scalar2=None, op0=AluOp.add)
                return off66

            def phase_b(c, off66):
                """Corner weights + pair-gather indices for chunk c."""
                # pixel-major offsets: offpx[q, jl, ch] (p = (c*8+jl)*128+q)
                offpx = pb.tile([128, JG, 18], f32, name=f"offpx_{c}",
                                tag=f"offpx_{c}")
                for hl in range(CR):
                    pt = ps_a.tile([64, 18], f32, tag="pa", name="offt")
                    nc.tensor.transpose(
                        pt[:], off66[:, hl * W2:hl * W2 + W],
                        ident_f32[0:18, 0:18])
                    nc.scalar.copy(
                        offpx[(hl % 2) * 64:(hl % 2) * 64 + 64, hl // 2, :],
                        pt[:])

                shp = [128, JG, 18]
                tl = {n: pb.tile(shp, f32, name=f"{n}_{c}", tag=n)
                      for n in ("PP", "FF", "II", "M0", "M1", "U0", "U1",
                                "T1", "ICY", "IC1")}
                w4 = pb.tile([128, 9, 4, JG], f32, name=f"w4_{c}", tag=f"w4_{c}")
                tb = pb.tile([128, JG, 9], f32, name=f"tb_{c}", tag="tb")
                idx16 = pb.tile([128, 9, 2, JG], i16, name=f"idx16_{c}",
                                tag="idx16")

                def ts(out, in0, s, op):
                    nc.vector.tensor_scalar(out=out, in0=in0, scalar1=s,
                                            scalar2=None, op0=op)

                PP, FF, II = tl["PP"], tl["FF"], tl["II"]
                M0, M1, U0, U1, T1 = (tl["M0"], tl["M1"], tl["U0"], tl["U1"],
                                      tl["T1"])
                ICY, IC1 = tl["ICY"], tl["IC1"]
                nc.vector.tensor_add(PP[:], offpx[:],
                                     kgrid[:, c * JG:(c + 1) * JG, :])
                ts(T1[:], PP[:], 0.5, AluOp.subtract)
                ts(T1[:], T1[:], MAGIC, AluOp.add)
                ts(II[:], T1[:], MAGIC, AluOp.subtract)    # II = floor(PP)
                nc.vector.tensor_sub(FF[:], PP[:], II[:])  # frac in [0,1)
                ts(M0[:], II[:], 0.0, AluOp.is_ge)
                ts(T1[:], II[:], 63.0, AluOp.is_le)
                nc.vector.tensor_mul(M0[:], M0[:], T1[:])
                ts(M1[:], II[:], -1.0, AluOp.is_ge)
                ts(T1[:], II[:], 62.0, AluOp.is_le)
                nc.vector.tensor_mul(M1[:], M1[:], T1[:])
                nc.vector.tensor_mul(T1[:], FF[:], M0[:])
                nc.vector.tensor_sub(U0[:], M0[:], T1[:])  # (1-f)*m0
                nc.vector.tensor_mul(U1[:], FF[:], M1[:])  # f*m1
                Us = (U0, U1)
                for i in range(2):
                    for xs in range(2):
                        nc.vector.tensor_mul(
                            w4[:, :, i * 2 + xs, :].rearrange("p k j -> p j k"),
                            Us[i][:, :, 0:18:2], Us[xs][:, :, 1:18:2])
                ts(ICY[:], II[:], 0.0, AluOp.max)
                ts(ICY[:], ICY[:], 63.0, AluOp.min)
                ts(IC1[:], II[:], 1.0, AluOp.add)
                ts(IC1[:], IC1[:], 0.0, AluOp.max)
                ts(IC1[:], IC1[:], 63.0, AluOp.min)
                for i, ic in enumerate((ICY, IC1)):
                    ts(tb[:], ic[:, :, 0:18:2], 64.0, AluOp.mult)
                    nc.vector.tensor_add(tb[:], tb[:], II[:, :, 1:18:2])
                    ts(tb[:], tb[:], -1.0, AluOp.max)
                    ts(tb[:], tb[:], 4095.0, AluOp.min)
                    ts(tb[:], tb[:], 1.0, AluOp.add)
                    nc.vector.tensor_copy(
                        idx16[:, :, i, :].rearrange("p k j -> p j k"), tb[:])

                # wrapped gather indices: position p -> (p%16, p//16).
                # SWDGE (gpsimd) keeps these small DMAs off the SP engine.
                idxw = pb.tile([128, 18, CH // 16], i16, name=f"idxw_{c}",
                               tag=f"idxw_{c}")
                for qh in range(8):
                    nc.sync.dma_start(
                        out=idxw[0:16, :, qh:CH // 16:8],
                        in_=idx16[qh * 16:(qh + 1) * 16, :, :, :].rearrange(
                            "p k i j -> p (k i) j"))
                # replicate to all 8 16-partition groups by doubling
                for m in (16, 32, 64):
                    nc.sync.dma_start(out=idxw[m:2 * m, :, :],
                                      in_=idxw[0:m, :, :])
                return w4, idxw

            # chunk-0 prep first (highest priority: first gathers gate all)
            prep = {}
            off66_0 = phase_a(0)

            # ---- xt_g build: PE row-transposes + DMA out (+ zero guards) ----
            xt_writes = []
            zg = pb.tile([4, 256], f16)
            nc.gpsimd.memset(zg[:], 0.0)
            xt_writes.append(nc.sync.dma_start(out=xt_g.ap()[0:1, :],
                                               in_=zg[0:1, :]))
            xt_writes.append(
                nc.sync.dma_start(out=xt_g.ap()[HW + 1:HW + 4, :], in_=zg[1:4, :]))
            # stage the whole transposed image, then one DMA to DRAM
            xt_sb = pb.tile([128, 32, 256], f16, name="xt_sb")
            for blk in range(32):
                h0 = blk * 2
                for cc in range(2):
                    for r in range(2):
                        pt = ps_tp.tile([64, 128], f16, tag="stp", name="xtp")
                        nc.tensor.transpose(
                            pt[:], x_pad[:, cc, 1 + h0 + r, 1:W + 1],
                            ident[:])
                        # split copybacks across ACT and DVE
                        eng = nc.scalar if (blk + cc) % 2 == 0 else nc.vector
                        if eng is nc.scalar:
                            nc.scalar.copy(
                                xt_sb[r * 64:(r + 1) * 64, blk,
                                      cc * 128:(cc + 1) * 128], pt[:])
                        else:
                            nc.vector.tensor_copy(
                                xt_sb[r * 64:(r + 1) * 64, blk,
                                      cc * 128:(cc + 1) * 128], pt[:])
            xt_writes.append(nc.sync.dma_start(
                out=xt_g.ap()[1:HW + 1, :].rearrange(
                    "(blk q) c -> q blk c", q=128),
                in_=xt_sb[:]))
            xt_fence = nc.sync.nop()
            for wdma in xt_writes:
                add_dep_helper(xt_fence.ins, wdma.ins, reason="xt_g RAW fence")

            prep[0] = phase_b(0, off66_0)

            # ================= main loop =================
            xt_win = AP(tensor=xt_g, offset=0, ap=[[256, NROWS - 1], [1, 512]])
            for ch in range(NCHUNK):
                # prep for the next chunk goes first: higher priority, so it
                # overlaps with this chunk's gather/compute pipeline
                if ch + 1 < NCHUNK:
                    prep[ch + 1] = phase_b(ch + 1, phase_a(ch + 1))
                w4, idxw = prep[ch]
                accs = [ps_acc.tile([128, 512], f32, tag=f"acc{a}",
                                    name=f"acc{a}") for a in range(4)]
                for k in range(9):
                    gs = []
                    for i in range(2):
                        g = gp.tile([128, JG, 512], f16, tag=f"g{i}",
                                    name=f"g{i}", bufs=4)
                        ginst = nc.gpsimd.dma_gather(
                            out_ap=g[:], in_ap=xt_win,
                            idxs_ap=idxw[:, k * 2 + i, :],
                            num_idxs=CH, num_idxs_reg=CH,
                            elem_size=512, elem_step=256,
                            transpose=False)
                        add_dep_helper(ginst.ins, xt_fence.ins,
                                       reason="xt_g RAW fence")
                        gs.append(g)
                    s_t = gp.tile([128, JG, 256], f16, tag="s", name="s",
                                  bufs=2)
                    sk = gp.tile([128, 2, CH], f16, tag="sk", name="sk",
                                 bufs=2)
                    for j in range(JG):
                        # first corner: alternate DVE/ACT to balance load
                        if j % 2 == 0:
                            nc.scalar.activation(
                                s_t[:, j, :], gs[0][:, j, 0:256],
                                mybir.ActivationFunctionType.Copy,
                                scale=w4[:, k, 0, j:j + 1])
                        else:
                            nc.vector.tensor_scalar(
                                out=s_t[:, j, :], in0=gs[0][:, j, 0:256],
                                scalar1=w4[:, k, 0, j:j + 1], scalar2=None,
                                op0=AluOp.mult)
                        for (gi, sl, corner) in ((0, 1, 1), (1, 0, 2), (1, 1, 3)):
                            nc.vector.scalar_tensor_tensor(
                                out=s_t[:, j, :],
                                in0=gs[gi][:, j, sl * 256:(sl + 1) * 256],
                                scalar=w4[:, k, corner, j:j + 1],
                                in1=s_t[:, j, :],
                                op0=AluOp.mult, op1=AluOp.add)
                        # transpose this j-group to channel-major right away
                        for cc in range(2):
                            ptp = ps_tp.tile([128, 128], f16, tag="stp",
                                             name="stp")
                            nc.tensor.transpose(
                                ptp[:], s_t[:, j, cc * 128:(cc + 1) * 128],
                                ident[:])
                            nc.scalar.copy(
                                sk[:, cc, j * 128:(j + 1) * 128], ptp[:])
                    for cc in range(2):
                        for o in range(2):
                            for sub in range(2):
                                nc.tensor.matmul(
                                    accs[o * 2 + sub],
                                    convw_sb[:, k * 2 + cc,
                                             o * 128:(o + 1) * 128],
                                    sk[:, cc, sub * 512:(sub + 1) * 512],
                                    start=(k == 0 and cc == 0),
                                    stop=(k == 8 and cc == 1))
                for o in range(2):
                    ob = op_pool.tile([128, CH], f32, tag=f"ob{o}",
                                      name=f"ob{o}")
                    for sub in range(2):
                        nc.scalar.copy(ob[:, sub * 512:(sub + 1) * 512],
                                       accs[o * 2 + sub][:])
                    nc.sync.dma_start(
                        out=y_out.ap()[o * 128:(o + 1) * 128,
                                       ch * CH:(ch + 1) * CH],
                        in_=ob[:])
    nc.compile()
    return nc


_CACHE = {}


def _get_nc():
    if "nc" not in _CACHE:
        nc = bacc.Bacc("TRN2", target_bir_lowering=False, debug=False,
                       num_devices=NCORES)
        _CACHE["nc"] = _emit(nc)
    return _CACHE["nc"]


def _host_tables():
    if "kgrid" in _CACHE:
        return _CACHE["kgrid"]
    q = np.arange(128)[:, None, None]
    j = np.arange(32)[None, :, None]
    c = np.arange(18)[None, None, :]
    p = j * 128 + q
    k = c // 2
    d = c % 2
    ky, kx = k // 3, k % 3
    grid = np.where(d == 0, p // W + ky - 1, p % W + kx - 1).astype(np.float32)
    _CACHE["kgrid"] = np.ascontiguousarray(grid)
    return _CACHE["kgrid"]


def _pack_weights(offset_w, offset_b, conv_w):
    # offw lhsT: [cc, c, k, j] = offset_w[j, cc*128+c, ky, kx]
    ow = offset_w.reshape(18, 2, 128, 9).transpose(1, 2, 3, 0)
    # convw lhsT: [c, (k,cc) chunk, o] = conv_w[o, cc*128+c, k]
    cw = conv_w.reshape(256, 2, 128, 9).transpose(2, 3, 1, 0)  # c, k, cc, o
    cw = cw.reshape(128, 18, 256)
    ob = offset_b.reshape(18, 1)
    return (np.ascontiguousarray(ow, np.float32),
            np.ascontiguousarray(ob, np.float32),
            np.ascontiguousarray(cw, np.float32))


def make_in_maps(x, offset_w, offset_b, conv_w):
    ow, ob, cw = _pack_weights(np.asarray(offset_w), np.asarray(offset_b),
                               np.asarray(conv_w))
    kg = _host_tables()
    x = np.asarray(x, np.float32)
    return [
        {"x": np.ascontiguousarray(x[b].reshape(C, H * W)),
         "offw": ow, "offb": ob, "convw": cw, "kgrid": kg}
        for b in range(B)
    ]


def kernel(x, offset_w, offset_b, conv_w):
    nc = _get_nc()
    in_maps = make_in_maps(x, offset_w, offset_b, conv_w)
    res = bass_utils.run_bass_kernel_spmd(nc, in_maps,
                                          core_ids=list(range(NCORES)))
    out = np.stack([np.asarray(res.results[b]["y"]).reshape(O, H, W)
                    for b in range(B)])
    return out.astype(np.float32)

